# revision 28
# baseline (speedup 1.0000x reference)
"""Trainium2 Bass kernel for nn_Attention_51092930953251.

GQA attention with KV-cache at start_pos=1920 (total T=2048), B=8, S=128,
H=32, KVH=8, D=128. The harness cache is all zeros, so positions
0..start_pos-1 contribute exactly exp(mask[s,t]) to the softmax denominator
(P0[s], host-known) and nothing to the numerator. Batch is sharded 1:1
across 8 cores.

v3 design (all fp16 on device, minimal instruction count):
  - host folds SCALE into q, casts q/k/v/mask to fp16
  - mask applied multiplicatively: p = exp(s) * exp(m) (both ~e^N(0,1),
    fp16-safe); exp on scalar engine, multiply on gpsimd
  - AV matmul has a ones column -> per-head row-sums land in PSUM with o
  - NO on-device softmax denominator: raw o + rowsum are copied fp16 to
    SBUF (vector engine) and shipped out; host adds P0 and normalizes
  - DMA dispatch is ~650ns of engine time per instruction, so loads are
    4 big chunks + stores are 4 group-pairs, all on the sync queue
"""

import math

import numpy as np

B, S, DIM, KV_DIM = 8, 128, 4096, 1024
H, KVH, D = 32, 8, 128
NREP = H // KVH  # 4
START = 1920
T = START + S  # 2048
SCALE = 1.0 / math.sqrt(D)
NCORES = 8
GW = D + NREP * S  # 640: one group's k (128) + q (512) columns
OGW = NREP * (D + 1)  # 516: one group's raw output (4 reps x (128+rowsum))

# tuning flags
N_WARM = 2  # PE wake-up matmuls
EM_ON_GPSIMD = True  # p~ * exp(mask) on gpsimd (else vector)

_BUILT = {}


def _build_nc(em_on_gpsimd=None):
    if em_on_gpsimd is None:
        em_on_gpsimd = EM_ON_GPSIMD
    import concourse.bacc as bacc
    import concourse.mybir as mybir
    import concourse.tile as tile

    f32 = mybir.dt.float32
    f16 = mybir.dt.float16
    AF = mybir.ActivationFunctionType
    ALU = mybir.AluOpType

    nc = bacc.Bacc(
        "TRN2", target_bir_lowering=False, debug=False, num_devices=NCORES
    )
    # kq = [d=128, g*(k_t'(128) | q_{r*S+s}(512))] fp16, partition-major so
    # each DMA moves multi-KB contiguous rows per partition
    kq_d = nc.dram_tensor("kq", [128, KVH * GW], f16, kind="ExternalInput")
    v_d = nc.dram_tensor("vones", [S, KVH * (D + 1)], f16, kind="ExternalInput")
    em_d = nc.dram_tensor("em4", [S, NREP * S], f16, kind="ExternalInput")
    # raw (unnormalized) output incl. rowsums, 2 groups per store
    out_d = nc.dram_tensor("out", [KVH // 2, S, 2 * OGW], f16, kind="ExternalOutput")

    with tile.TileContext(nc) as tc:
        with (
            tc.tile_pool(name="big", bufs=1) as big,
            tc.tile_pool(name="work", bufs=4) as work,
            tc.tile_pool(name="ps_s", bufs=3, space="PSUM") as ps_s,
            tc.tile_pool(name="ps_o", bufs=5, space="PSUM") as ps_o,
        ):
            kq_sb = big.tile([128, KVH * GW], f16, tag="kq")
            v_sb = big.tile([S, KVH * (D + 1)], f16, tag="v")
            em_sb = big.tile([S, NREP * S], f16, tag="em")
            og_sb = big.tile([S, KVH * OGW], f16, tag="og")

            def load_kq(g0, g1, eng):
                eng.dma_start(
                    kq_sb[:, g0 * GW : g1 * GW],
                    kq_d.ap()[:, g0 * GW : g1 * GW],
                )

            # loads split across both HWDGE queues in need-order.
            # sync: kq in 3 chunks (0-1 first so the PE starts early);
            # scalar: mask, v; warm_exp (ACT-table preload) before the
            # scalar queue's kq tail.
            load_kq(0, 1, nc.sync)
            nc.scalar.dma_start(em_sb[:, :], em_d.ap())
            load_kq(1, 4, nc.sync)
            nc.scalar.dma_start(v_sb[:, :], v_d.ap())

            # PE wake-up; memset on vector (idle at startup), results
            # discarded
            warm_sb = big.tile([128, 128], f16, tag="warm")
            warmx_sb = big.tile([128, 1], f16, tag="warmexp")
            nc.vector.memset(warm_sb[:, :], 0.0)
            nc.scalar.activation(warmx_sb[:, :], warm_sb[:, 0:1], AF.Exp)
            load_kq(4, 8, nc.scalar)
            warm_ps = ps_s.tile([128, NREP * 128], f32, tag="sT")
            for _ in range(N_WARM):
                nc.tensor.matmul(
                    warm_ps[:, 0:128], warm_sb[:, :], warm_sb[:, :]
                )

            def emit_s(g):
                # S^T: out [t', 4s] f32
                sT_ps = ps_s.tile([128, NREP * 128], f32, tag="sT")
                nc.tensor.matmul(
                    sT_ps[:, :],
                    kq_sb[:, g * GW : g * GW + D],
                    kq_sb[:, g * GW + D : (g + 1) * GW],
                )
                return sT_ps

            def emit_p(g, sT_ps):
                # p~ = exp(s) on scalar; p = p~ * exp(mask). GpSimd's TT is
                # ~1.15us vs vector's ~0.43us, so gpsimd only relieves
                # vector for mid-pipeline groups (1, 3); the late groups
                # stay on vector (idle by then) to keep the tail chain
                # exp(7)->em(7)->AV(7) short
                pt_sb = work.tile([128, NREP * 128], f16, tag="pt")
                nc.scalar.activation(pt_sb[:, :], sT_ps[:, :], AF.Exp)
                p_sb = work.tile([128, NREP * 128], f16, tag="p")
                eng = nc.gpsimd if (em_on_gpsimd and g in (1, 3)) else nc.vector
                eng.tensor_tensor(
                    p_sb[:, :], pt_sb[:, :], em_sb[:, :], ALU.mult
                )
                return p_sb

            def emit_av(g, p_sb):
                # AV with ones column, two heads packed per PSUM tile
                o_tiles = []
                for j in range(2):
                    o_ps = ps_o.tile([128, 2 * (D + 1)], f32, tag="o")
                    o_tiles.append(o_ps)
                    for i in range(2):
                        r = 2 * j + i
                        nc.tensor.matmul(
                            o_ps[:, i * (D + 1) : (i + 1) * (D + 1)],
                            p_sb[:, r * 128 : (r + 1) * 128],
                            v_sb[:, g * (D + 1) : (g + 1) * (D + 1)],
                        )
                return o_tiles

            def emit_copies(g, o_tiles):
                # raw o (+rowsum cols) PSUM f32 -> SBUF fp16. Per group
                # PAIR the 4 copies + 2 em-mults + 2 exps are spread as:
                # scalar 2 exp + 1 copy, vector 3 copies + 1 em, gpsimd
                # 1 em -> ~925ns/group all-engine balance
                nc.vector.tensor_scalar_add(
                    og_sb[:, g * OGW : g * OGW + 258],
                    o_tiles[0][:, :],
                    0.0,
                )
                if g % 2 == 1:
                    # scalar Copy shares the Exp table slot (no reload)
                    nc.scalar.activation(
                        og_sb[:, g * OGW + 258 : (g + 1) * OGW],
                        o_tiles[1][:, :],
                        AF.Copy,
                    )
                else:
                    nc.vector.tensor_scalar_add(
                        og_sb[:, g * OGW + 258 : (g + 1) * OGW],
                        o_tiles[1][:, :],
                        0.0,
                    )
                # stores: pairs early (fewer dispatches), singles for the
                # last two groups (shorter tail), alternating queues
                if g in (1, 5):
                    nc.sync.dma_start(
                        out_d.ap()[g // 2],
                        og_sb[:, (g - 1) * OGW : (g + 1) * OGW],
                    )
                elif g == 3:
                    nc.scalar.dma_start(
                        out_d.ap()[1],
                        og_sb[:, 2 * OGW : 4 * OGW],
                    )
                elif g == 6:
                    nc.scalar.dma_start(
                        out_d.ap()[3][:, 0:OGW],
                        og_sb[:, 6 * OGW : 7 * OGW],
                    )
                elif g == 7:
                    nc.sync.dma_start(
                        out_d.ap()[3][:, OGW:],
                        og_sb[:, 7 * OGW :],
                    )

            # software pipeline: S runs 3 groups ahead; next group's exp is
            # issued before this group's PSUM->SBUF copies
            sT = {0: emit_s(0), 1: emit_s(1)}
            pT = {0: emit_p(0, sT.pop(0))}
            sT[2] = emit_s(2)
            prev = None
            for g in range(KVH):
                o_tiles = emit_av(g, pT.pop(g))
                if g + 1 < KVH:
                    pT[g + 1] = emit_p(g + 1, sT.pop(g + 1))
                if g + 3 < KVH:
                    sT[g + 3] = emit_s(g + 3)
                if prev is not None:
                    emit_copies(*prev)
                prev = (g, o_tiles)
            emit_copies(*prev)

    nc.compile()
    return nc


def _get_nc():
    key = ("v3", EM_ON_GPSIMD, N_WARM)
    if key not in _BUILT:
        _BUILT[key] = _build_nc(EM_ON_GPSIMD)
    return _BUILT[key]


def _reference_fallback(q, k, v, start_pos, mask, cache_k, cache_v):
    b, s, _ = q.shape
    start_pos = int(start_pos)
    t = start_pos + s
    xq = q.reshape(b, s, H, D).astype(np.float32)
    xk = k.reshape(b, s, KVH, D).astype(np.float32)
    xv = v.reshape(b, s, KVH, D).astype(np.float32)
    ck = np.array(cache_k[:b, :t], dtype=np.float32, copy=True)
    cv = np.array(cache_v[:b, :t], dtype=np.float32, copy=True)
    ck[:, start_pos:t] = xk
    cv[:, start_pos:t] = xv
    xqg = xq.reshape(b, s, KVH, NREP, D)
    scores = np.einsum("bsgrd,btgd->bgrst", xqg, ck) * SCALE
    scores = scores + np.asarray(mask, dtype=np.float32)[:, :, None]
    scores -= scores.max(axis=-1, keepdims=True)
    p = np.exp(scores)
    p /= p.sum(axis=-1, keepdims=True)
    out = np.einsum("bgrst,btgd->bsgrd", p, cv)
    return out.reshape(b, s, H * D).astype(np.float32)


def kernel(q, k, v, start_pos, freqs_cis, mask, cache_k, cache_v):
    q = np.asarray(q, dtype=np.float32)
    k = np.asarray(k, dtype=np.float32)
    v = np.asarray(v, dtype=np.float32)
    mask = np.asarray(mask, dtype=np.float32)
    sp = int(start_pos)

    fast_ok = (
        sp == START
        and q.shape == (B, S, DIM)
        and k.shape == (B, S, KV_DIM)
        and v.shape == (B, S, KV_DIM)
        and mask.shape == (1, 1, S, T)
        and not np.asarray(cache_k)[:B, :START].any()
        and not np.asarray(cache_v)[:B, :START].any()
    )
    if not fast_ok:
        return _reference_fallback(q, k, v, sp, mask, cache_k, cache_v)

    from concourse.bass_utils import run_bass_kernel_spmd

    nc = _get_nc()

    m2d = mask[0, 0]  # [S, T]
    p0 = np.exp(m2d[:, :START]).sum(axis=1)  # [s]
    em = np.exp(m2d[:, START:].T)  # [t', s]
    em4 = np.ascontiguousarray(np.tile(em, (1, NREP)), np.float16)

    # host layout prep: kq[b, g] = [d, k_t' | SCALE*q_{r*S+s}]
    kt = k.reshape(B, S, KVH, D).transpose(0, 2, 3, 1)  # [B, g, d, t']
    qt = (q * SCALE).reshape(B, S, KVH, NREP, D).transpose(0, 2, 4, 3, 1)
    kq = np.empty((B, 128, KVH, GW), dtype=np.float16)  # partition-major
    kq[:, :, :, :D] = kt.transpose(0, 2, 1, 3)
    kq[:, :, :, D:] = qt.reshape(B, KVH, 128, NREP * S).transpose(0, 2, 1, 3)
    kq = kq.reshape(B, 128, KVH * GW)
    vones = np.empty((B, S, KVH, D + 1), dtype=np.float16)
    vones[..., :D] = v.reshape(B, S, KVH, D)
    vones[..., D] = 1.0
    vones = vones.reshape(B, S, KVH * (D + 1))

    in_maps = [
        {"kq": kq[b], "vones": vones[b], "em4": em4}
        for b in range(B)
    ]
    res = run_bass_kernel_spmd(nc, in_maps, list(range(NCORES)))
    # device out: [4, s, 2*516] fp16 raw (o | rowsum); host normalizes
    out = np.empty((B, S, KVH, NREP, D), dtype=np.float32)
    for b in range(B):
        raw = res.results[b]["out"].astype(np.float32)
        o5 = raw.reshape(KVH // 2, S, 2, NREP, D + 1)  # [pr, s, half, r, d+1]
        denom = o5[..., D] + p0[None, :, None, None]  # [pr, s, half, r]
        oo = o5[..., :D] / denom[..., None]
        # group g = 2*pr + half
        out[b] = oo.transpose(1, 0, 2, 3, 4).reshape(S, KVH, NREP, D)
    return np.ascontiguousarray(out.reshape(B, S, DIM))
